# revision 17
# baseline (speedup 1.0000x reference)
"""GAT (4-layer) + GraphNorm + PReLU on 8 Trainium2 NeuronCores.

Strategy (dst-sharded message passing):
  - Host: append self-loops, sort edges by destination, shard destinations
    contiguously across 8 cores (2048 nodes each), group each core's edges by
    128-dst block, pad each block to a uniform number of 128-edge chunks.
  - Device (SPMD, same program all cores, per-core input data):
      per layer:
        A) node phase: xl = x @ W (PE, fp32r), a_src/a_dst via a host-packed
           [F, 2H] attention matrix (PE), write records [xl | a_src] to DRAM.
        B) AllGather records -> every core holds all 16384 node records.
        C) edge phase: per 128-edge chunk, indirect-DMA gather the source
           records, build a selection matrix S[e,d] = (dstloc[e] == d) via
           iota + is_equal, and matmul-accumulate
              out[d,:]  += S^T @ (w ⊙ G)      (PSUM, fp32 accum)
              s[d,h]    += S^T @ w            (softmax denominator)
           where w = exp(leakyrelu(a_src[src] + a_dst[dst])).  Softmax
           normalization is folded to the block end: out = out / s + bias.
        D) GraphNorm stats (column sums via ones-matmul) -> tiny AllReduce ->
           per-channel affine + PReLU -> next layer input.
  - alpha (attention coefficients of the last layer) are computed per chunk as
    w * (1/s)[dst] and written in sorted-edge order; the host scatters them
    back to the original edge order (a pure permutation).
"""

import numpy as np

# Problem constants (from the nn_GATs problem spec).
N = 16384
E = 262144
H = 4
C = 64
F = H * C  # 256
D0 = 128
NCORES = 8
P = 128
NSHARD = N // NCORES  # 2048
NBLK = NSHARD // P  # 16 dst blocks per core
RECW = F + 64  # record row: [xl(256) | a_src(4) | pad] -> 1280B, %256 == 0
NEG_SLOPE = 0.2
GN_EPS = 1e-5

_KERNEL_CACHE = {}


# --------------------------------------------------------------------------
# Host-side graph planning (structure only; no feature math).
# --------------------------------------------------------------------------
def _plan_edges(edge_index):
    src_all = np.concatenate(
        [np.asarray(edge_index[0], np.int64), np.arange(N, dtype=np.int64)]
    )
    dst_all = np.concatenate(
        [np.asarray(edge_index[1], np.int64), np.arange(N, dtype=np.int64)]
    )
    e_tot = src_all.shape[0]
    order = np.argsort(dst_all, kind="stable")
    s_src = src_all[order]
    s_dst = dst_all[order]

    key = (s_dst // NSHARD) * NBLK + (s_dst % NSHARD) // P  # (core, block) group
    dloc = (s_dst % P).astype(np.float32)
    cnts = np.bincount(key, minlength=NCORES * NBLK)
    b_chunks = int(np.ceil(cnts.max() / P))
    cpb = b_chunks * P  # padded capacity per block

    grp_starts = np.zeros(NCORES * NBLK, np.int64)
    grp_starts[1:] = np.cumsum(cnts)[:-1]
    pos_in_grp = np.arange(e_tot, dtype=np.int64) - grp_starts[key]
    slot = key * cpb + pos_in_grp

    idx_flat = np.zeros(NCORES * NBLK * cpb, np.int32)
    dloc_flat = np.full(NCORES * NBLK * cpb, 200.0, np.float32)
    orig_flat = np.full(NCORES * NBLK * cpb, -1, np.int64)
    idx_flat[slot] = s_src.astype(np.int32)
    dloc_flat[slot] = dloc
    orig_flat[slot] = order

    # Device layout [P, NBLK * b_chunks]: [p, b*b_chunks + c] = slot (b, c, p).
    def to_dev(a):
        return np.ascontiguousarray(
            a.reshape(NCORES, NBLK, b_chunks, P).transpose(0, 3, 1, 2).reshape(
                NCORES, P, NBLK * b_chunks
            )
        )

    idx_dev = to_dev(idx_flat)
    dloc_dev = to_dev(dloc_flat)
    orig = orig_flat.reshape(NCORES, NBLK, b_chunks, P)
    return b_chunks, idx_dev, dloc_dev, orig, src_all, dst_all


# --------------------------------------------------------------------------
# Device kernel builder (parametric so it can be simulated at small sizes).
# --------------------------------------------------------------------------
def build_kernel(n_cores, n_nodes, b_chunks, d0=D0, n_layers=4, enable_asserts=False,
                 edge_dtype="float16", skip_collective=False):
    import concourse.bacc as bacc
    import concourse.bass as bass
    import concourse.tile as tile
    from concourse import mybir
    from concourse.masks import make_identity

    f32 = mybir.dt.float32
    f16 = getattr(mybir.dt, edge_dtype) if edge_dtype else mybir.dt.float32
    i32 = mybir.dt.int32
    Alu = mybir.AluOpType
    Act = mybir.ActivationFunctionType

    nshard = n_nodes // n_cores
    nblk = nshard // P
    ncols = nblk * b_chunks

    nc = bacc.Bacc(
        "TRN2",
        target_bir_lowering=False,
        debug=False,
        enable_asserts=enable_asserts,
        num_devices=n_cores,
    )

    # I/O -------------------------------------------------------------------
    x0_d = nc.dram_tensor("x0", [nshard, d0], f32, kind="ExternalInput").ap()
    idx_d = nc.dram_tensor("idx", [P, ncols], i32, kind="ExternalInput").ap()
    dloc_d = nc.dram_tensor("dloc", [P, ncols], f32, kind="ExternalInput").ap()
    w0_d = nc.dram_tensor("W0", [d0, F], f32, kind="ExternalInput").ap()
    wr_d = nc.dram_tensor("Wr", [max(n_layers - 1, 1), F, F], f32, kind="ExternalInput").ap()
    attm_d = nc.dram_tensor("attm", [n_layers, F, 2 * H], f32, kind="ExternalInput").ap()
    bias_d = nc.dram_tensor("bias", [n_layers, F], f32, kind="ExternalInput").ap()
    gnw_d = nc.dram_tensor("gnw", [n_layers, F], f32, kind="ExternalInput").ap()
    gnb_d = nc.dram_tensor("gnb", [n_layers, F], f32, kind="ExternalInput").ap()
    gnms_d = nc.dram_tensor("gnms", [n_layers, F], f32, kind="ExternalInput").ap()
    prelu_d = nc.dram_tensor("prelu", [n_layers, 1], f32, kind="ExternalInput").ap()

    xout_d = nc.dram_tensor("xout", [nshard, F], f32, kind="ExternalOutput").ap()
    aout_d = nc.dram_tensor(
        "aout", [nblk, P, b_chunks * H], f32, kind="ExternalOutput"
    ).ap()

    rg = [list(range(n_cores))]

    with tile.TileContext(nc) as tc:
        with (
            tc.tile_pool(name="dram", bufs=1, space="DRAM") as dpool,
            tc.tile_pool(name="const", bufs=1) as cpool,
            tc.tile_pool(name="params", bufs=2) as ppool,
            tc.tile_pool(name="xbuf", bufs=2) as xpool,
            tc.tile_pool(name="edge", bufs=1) as epool,
            tc.tile_pool(name="gbuf", bufs=4) as gpool,
            tc.tile_pool(name="work", bufs=3) as wpool,
            tc.tile_pool(name="sttl", bufs=3) as stpool_small,
            tc.tile_pool(name="sttb", bufs=b_chunks + 2) as stpool_big,
            # PSUM: 8 banks total. One shared transient tag (4 banks) +
            # agg (1) + sden (1) + stats (1) = 7.
            tc.tile_pool(name="psA", bufs=4, space="PSUM") as psA,
            tc.tile_pool(name="psO", bufs=1, space="PSUM") as psO,
            tc.tile_pool(name="psS", bufs=1, space="PSUM") as psS,
            tc.tile_pool(name="psst", bufs=1, space="PSUM") as psst,
        ):
            # DRAM internals, one per layer: Shared collective outputs must
            # each have a single writing instruction.
            shared_as = "Shared" if n_cores > 4 else "Local"
            rec_shard = [
                dpool.tile([nshard, RECW], f16, name=f"rec_shard{i}")
                for i in range(n_layers)
            ]
            rec_full = [
                dpool.tile(
                    [n_nodes, RECW], f16, addr_space=shared_as, name=f"rec_full{i}"
                )
                for i in range(n_layers)
            ]
            st_in = [
                dpool.tile([1, 2 * F], f32, name=f"st_in{i}") for i in range(n_layers)
            ]
            st_out = [
                dpool.tile([1, 2 * F], f32, addr_space=shared_as, name=f"st_out{i}")
                for i in range(n_layers)
            ]
            rowab_bounce = [
                dpool.tile([1, 2 * F], f32, name=f"rowab{i}") for i in range(n_layers)
            ]

            # Constants.
            iota_row = cpool.tile([P, P], f32, name="iota_row")
            nc.gpsimd.iota(
                iota_row[:],
                pattern=[[1, P]],
                base=0,
                channel_multiplier=0,
                allow_small_or_imprecise_dtypes=True,
            )
            identity = cpool.tile([P, P], f32, name="identity")
            make_identity(nc, identity[:])
            identity16 = cpool.tile([P, P], f16, name="identity16")
            nc.vector.tensor_copy(out=identity16[:], in_=identity[:])
            ones_col = cpool.tile([P, 1], f32, name="ones_col")
            nc.vector.memset(ones_col[:], 1.0)

            # Edge structure (resident all layers).
            idx_sb = epool.tile([P, ncols], i32, name="idx_sb")
            dloc_sb = epool.tile([P, ncols], f32, name="dloc_sb")
            nc.sync.dma_start(out=idx_sb[:], in_=idx_d[:])
            nc.sync.dma_start(out=dloc_sb[:], in_=dloc_d[:])

            # Initial x tiles.
            xcur = []
            for t in range(nblk):
                xt = xpool.tile([P, F], f32, tag=f"xcur{t}", name=f"xcur_{t}")
                nc.sync.dma_start(
                    out=xt[:, :d0], in_=x0_d[t * P : (t + 1) * P, :]
                )
                xcur.append(xt)

            inv_n = 1.0 / float(n_nodes)

            for l in range(n_layers):
                din_l = d0 if l == 0 else F
                ktiles = din_l // P
                par = l

                # ---- layer parameter tiles ----
                w_sb = []
                for kk in range(ktiles):
                    wt = ppool.tile([P, F], f32, tag=f"w{kk}", name=f"w_l{l}_{kk}")
                    if l == 0:
                        nc.sync.dma_start(out=wt[:], in_=w0_d[kk * P : (kk + 1) * P, :])
                    else:
                        nc.sync.dma_start(
                            out=wt[:], in_=wr_d[l - 1, kk * P : (kk + 1) * P, :]
                        )
                    w_sb.append(wt)
                attm_sb = []
                for kk in range(ktiles):
                    at = ppool.tile([P, 2 * H], f32, tag=f"attm{kk}", name=f"attm_l{l}_{kk}")
                    nc.sync.dma_start(
                        out=at[:], in_=attm_d[l, kk * P : (kk + 1) * P, :]
                    )
                    attm_sb.append(at)
                bias_b = ppool.tile([P, F], f32, tag="bias_b", name=f"bias_l{l}")
                nc.gpsimd.dma_start(
                    out=bias_b[:], in_=bias_d[l : l + 1, :].to_broadcast([P, F])
                )
                gn_rows = ppool.tile([1, 3 * F], f32, tag="gn_rows", name=f"gn_l{l}")
                nc.sync.dma_start(out=gn_rows[:, 0:F], in_=gnw_d[l : l + 1, :])
                nc.sync.dma_start(out=gn_rows[:, F : 2 * F], in_=gnb_d[l : l + 1, :])
                nc.sync.dma_start(out=gn_rows[:, 2 * F : 3 * F], in_=gnms_d[l : l + 1, :])
                pr_col = ppool.tile([P, 1], f32, tag="pr_col", name=f"pr_l{l}")
                nc.gpsimd.dma_start(
                    out=pr_col[:], in_=prelu_d[l : l + 1, :].to_broadcast([P, 1])
                )
                pr_om = ppool.tile([P, 1], f32, tag="pr_om", name=f"prom_l{l}")
                # 1 - a
                nc.vector.tensor_scalar(
                    pr_om[:], pr_col[:], -1.0, 1.0, Alu.mult, Alu.add
                )

                # ---- Phase A: node transform ----
                a_dst_all = xpool.tile([P, nblk * H], f16, tag="adst", name=f"adst_l{l}")
                for t in range(nblk):
                    xt = xcur[t]
                    xT = []
                    for kk in range(ktiles):
                        trp = psA.tile([P, P], f32, tag="ps", name=f"trp_l{l}_{t}_{kk}")
                        nc.tensor.transpose(
                            out=trp[:],
                            in_=xt[:, kk * P : (kk + 1) * P],
                            identity=identity[:],
                        )
                        xTs = wpool.tile([P, P], f32, tag="xT", name=f"xT_l{l}_{t}_{kk}")
                        nc.vector.tensor_copy(out=xTs[:], in_=trp[:])
                        xT.append(xTs)
                    xlp = psA.tile([P, F], f32, tag="ps", name=f"xlp_l{l}_{t}")
                    for kk in range(ktiles):
                        nc.tensor.matmul(
                            out=xlp[:],
                            lhsT=xT[kk][:],
                            rhs=w_sb[kk][:],
                            start=(kk == 0),
                            stop=(kk == ktiles - 1),
                        )
                    ap_ps = psA.tile([P, 2 * H], f32, tag="ps", name=f"aps_l{l}_{t}")
                    for kk in range(ktiles):
                        nc.tensor.matmul(
                            out=ap_ps[:],
                            lhsT=xT[kk][:],
                            rhs=attm_sb[kk][:],
                            start=(kk == 0),
                            stop=(kk == ktiles - 1),
                        )
                    xl_sb = wpool.tile([P, F], f16, tag="xl_sb", name=f"xlsb_l{l}_{t}")
                    nc.vector.tensor_copy(out=xl_sb[:], in_=xlp[:])
                    asrc_t = wpool.tile([P, RECW - F], f16, tag="asrc", name=f"asrc_l{l}_{t}")
                    nc.vector.memset(asrc_t[:, H:], 0.0)
                    nc.vector.tensor_copy(out=asrc_t[:, 0:H], in_=ap_ps[:, 0:H])
                    nc.vector.tensor_copy(
                        out=a_dst_all[:, t * H : (t + 1) * H], in_=ap_ps[:, H : 2 * H]
                    )
                    nc.sync.dma_start(
                        out=rec_shard[par][t * P : (t + 1) * P, 0:F], in_=xl_sb[:]
                    )
                    nc.sync.dma_start(
                        out=rec_shard[par][t * P : (t + 1) * P, F:RECW],
                        in_=asrc_t[:],
                    )

                # ---- Phase B: AllGather records ----
                if skip_collective:
                    nc.sync.dma_start(
                        out=rec_full[par][nshard : 2 * nshard, :],
                        in_=rec_shard[par][:],
                    )
                else:
                    nc.gpsimd.collective_compute(
                        "AllGather",
                        Alu.bypass,
                        replica_groups=rg,
                        ins=[rec_shard[par].opt()],
                        outs=[rec_full[par].opt()],
                    )

                # ---- Phase C: edge aggregation ----
                stp0 = psst.tile([1, F], f32, tag="st0", name=f"stats0_l{l}")
                stp1 = psst.tile([1, F], f32, tag="st1", name=f"stats1_l{l}")
                last = l == n_layers - 1
                st_keep_pool = stpool_big if last else stpool_small
                w_keep_pool = stpool_big if last else stpool_small
                xnew = []
                for b in range(nblk):
                    agg = psO.tile([P, F], f32, tag="agg", name=f"agg_l{l}_{b}")
                    sden = psS.tile([P, H], f32, tag="sden", name=f"sden_l{l}_{b}")
                    st_tiles = []
                    w_tiles = []
                    for cch in range(b_chunks):
                        col = b * b_chunks + cch
                        g = gpool.tile([P, RECW], f16, tag="G", name=f"g_l{l}_{b}_{cch}")
                        nc.gpsimd.indirect_dma_start(
                            out=g[:],
                            out_offset=None,
                            in_=rec_full[par][:],
                            in_offset=bass.IndirectOffsetOnAxis(
                                ap=idx_sb[:, col : col + 1], axis=0
                            ),
                        )
                        s_t = wpool.tile([P, P], f16, tag="S", name=f"s_l{l}_{b}_{cch}")
                        nc.vector.tensor_scalar(
                            s_t[:],
                            iota_row[:],
                            dloc_sb[:, col : col + 1],
                            None,
                            Alu.is_equal,
                        )
                        stps = psA.tile([P, P], f16, tag="ps", name=f"stp_l{l}_{b}_{cch}")
                        nc.tensor.transpose(
                            out=stps[:], in_=s_t[:], identity=identity16[:]
                        )
                        st_t = st_keep_pool.tile(
                            [P, P], f16, tag="ST", name=f"st_l{l}_{b}_{cch}"
                        )
                        nc.vector.tensor_copy(out=st_t[:], in_=stps[:])
                        # a_dst per edge
                        adst_e = psA.tile([P, H], f32, tag="ps", name=f"adste_l{l}_{b}_{cch}")
                        nc.tensor.matmul(
                            out=adst_e[:],
                            lhsT=st_t[:],
                            rhs=a_dst_all[:, b * H : (b + 1) * H],
                            start=True,
                            stop=True,
                        )
                        asrc_e = wpool.tile([P, H], f32, tag="asrc_e", name=f"asrce_l{l}_{b}_{cch}")
                        nc.vector.tensor_copy(out=asrc_e[:], in_=g[:, F : F + H])
                        epre = wpool.tile([P, H], f32, tag="epre", name=f"epre_l{l}_{b}_{cch}")
                        nc.vector.tensor_tensor(
                            out=epre[:],
                            in0=adst_e[:],
                            in1=asrc_e[:],
                            op=Alu.add,
                        )
                        lr = wpool.tile([P, H], f32, tag="lr", name=f"lr_l{l}_{b}_{cch}")
                        nc.scalar.activation(
                            lr[:], epre[:], Act.Relu, scale=1.0 - NEG_SLOPE
                        )
                        wpre = wpool.tile([P, H], f32, tag="wpre", name=f"wpre_l{l}_{b}_{cch}")
                        nc.vector.scalar_tensor_tensor(
                            out=wpre[:],
                            in0=epre[:],
                            scalar=NEG_SLOPE,
                            in1=lr[:],
                            op0=Alu.mult,
                            op1=Alu.add,
                        )
                        wt_pool = w_keep_pool if last else wpool
                        w_t = wt_pool.tile([P, H], f32, tag="wv", name=f"w_l{l}_{b}_{cch}")
                        nc.scalar.activation(w_t[:], wpre[:], Act.Exp)
                        w16 = w_keep_pool.tile([P, H], f16, tag="w16", name=f"w16_l{l}_{b}_{cch}")
                        nc.vector.tensor_copy(out=w16[:], in_=w_t[:])
                        # messages M = G * w (per head), on ACT via scale AP
                        m_t = wpool.tile([P, F], f16, tag="M", name=f"m_l{l}_{b}_{cch}")
                        for h in range(H):
                            nc.scalar.activation(
                                m_t[:, h * C : (h + 1) * C],
                                g[:, h * C : (h + 1) * C],
                                Act.Identity,
                                scale=w_t[:, h : h + 1],
                            )
                        nc.tensor.matmul(
                            out=agg[:],
                            lhsT=s_t[:],
                            rhs=m_t[:],
                            start=(cch == 0),
                            stop=(cch == b_chunks - 1),
                        )
                        nc.tensor.matmul(
                            out=sden[:],
                            lhsT=s_t[:],
                            rhs=w16[:],
                            start=(cch == 0),
                            stop=(cch == b_chunks - 1),
                        )
                        st_tiles.append(st_t)
                        w_tiles.append(w_t)

                    # block finalize
                    r_t = wpool.tile([P, H], f32, tag="r", name=f"r_l{l}_{b}")
                    nc.vector.tensor_scalar(
                        r_t[:], sden[:], 1e-16, None, Alu.add
                    )
                    nc.vector.reciprocal(r_t[:], r_t[:])
                    xn = xpool.tile([P, F], f32, tag=f"xnew{b}", name=f"xnew_l{l}_{b}")
                    for h in range(H):
                        nc.vector.tensor_scalar(
                            xn[:, h * C : (h + 1) * C],
                            agg[:, h * C : (h + 1) * C],
                            r_t[:, h : h + 1],
                            None,
                            Alu.mult,
                        )
                    nc.vector.tensor_tensor(
                        out=xn[:], in0=xn[:], in1=bias_b[:], op=Alu.add
                    )
                    sq = wpool.tile([P, F], f32, tag="sq", name=f"sq_l{l}_{b}")
                    nc.scalar.activation(sq[:], xn[:], Act.Square)
                    nc.tensor.matmul(
                        out=stp0[:],
                        lhsT=ones_col[:],
                        rhs=xn[:],
                        start=(b == 0),
                        stop=(b == nblk - 1),
                    )
                    nc.tensor.matmul(
                        out=stp1[:],
                        lhsT=ones_col[:],
                        rhs=sq[:],
                        start=(b == 0),
                        stop=(b == nblk - 1),
                    )
                    xnew.append(xn)

                    if last:
                        r16 = wpool.tile([P, H], f16, tag="r16", name=f"r16_{b}")
                        nc.vector.tensor_copy(out=r16[:], in_=r_t[:])
                        alpha_blk = wpool.tile(
                            [P, b_chunks * H], f32, tag="ablk", name=f"ablk_{b}"
                        )
                        for cch in range(b_chunks):
                            re_ps = psA.tile([P, H], f32, tag="ps", name=f"re_{b}_{cch}")
                            nc.tensor.matmul(
                                out=re_ps[:],
                                lhsT=st_tiles[cch][:],
                                rhs=r16[:],
                                start=True,
                                stop=True,
                            )
                            re_sb = wpool.tile([P, H], f32, tag="re_sb", name=f"resb_{b}_{cch}")
                            nc.vector.tensor_copy(out=re_sb[:], in_=re_ps[:])
                            nc.vector.tensor_tensor(
                                out=alpha_blk[:, cch * H : (cch + 1) * H],
                                in0=re_sb[:],
                                in1=w_tiles[cch][:],
                                op=Alu.mult,
                            )
                        nc.sync.dma_start(out=aout_d[b], in_=alpha_blk[:])

                # ---- Phase D: GraphNorm stats AllReduce + norm + PReLU ----
                st_sb = wpool.tile([1, 2 * F], f32, tag="st_sb", name=f"stsb_l{l}")
                nc.vector.tensor_copy(out=st_sb[:, 0:F], in_=stp0[:])
                nc.vector.tensor_copy(out=st_sb[:, F : 2 * F], in_=stp1[:])
                nc.sync.dma_start(out=st_in[par][:], in_=st_sb[:])
                nc.gpsimd.collective_compute(
                    "AllReduce",
                    Alu.add,
                    replica_groups=rg,
                    ins=[st_in[par].opt()],
                    outs=[st_out[par].opt()],
                )
                st_rb = wpool.tile([1, 2 * F], f32, tag="st_rb", name=f"strb_l{l}")
                nc.sync.dma_start(out=st_rb[:], in_=st_out[par][:])

                # per-channel affine rows (partition 0)
                rows = wpool.tile([1, 8 * F], f32, tag="rows", name=f"rows_l{l}")
                mean = rows[:, 0:F]
                m2 = rows[:, F : 2 * F]
                t1 = rows[:, 2 * F : 3 * F]
                t2 = rows[:, 3 * F : 4 * F]
                var = rows[:, 4 * F : 5 * F]
                sd = rows[:, 5 * F : 6 * F]
                arow = rows[:, 6 * F : 7 * F]
                brow = rows[:, 7 * F : 8 * F]
                gnw_r = gn_rows[:, 0:F]
                gnb_r = gn_rows[:, F : 2 * F]
                gnms_r = gn_rows[:, 2 * F : 3 * F]
                nc.vector.tensor_scalar(mean, st_rb[:, 0:F], inv_n, None, Alu.mult)
                nc.vector.tensor_scalar(m2, st_rb[:, F : 2 * F], inv_n, None, Alu.mult)
                nc.vector.tensor_tensor(out=t1, in0=mean, in1=gnms_r, op=Alu.mult)
                nc.vector.scalar_tensor_tensor(
                    out=t2, in0=mean, scalar=2.0, in1=t1, op0=Alu.mult, op1=Alu.subtract
                )
                nc.vector.tensor_tensor(out=var, in0=t1, in1=t2, op=Alu.mult)
                nc.vector.tensor_tensor(out=var, in0=m2, in1=var, op=Alu.subtract)
                nc.vector.tensor_scalar(var, var, GN_EPS, None, Alu.add)
                nc.scalar.activation(sd, var, Act.Sqrt)
                nc.vector.reciprocal(sd, sd)
                nc.vector.tensor_tensor(out=arow, in0=gnw_r, in1=sd, op=Alu.mult)
                nc.vector.tensor_tensor(out=brow, in0=t1, in1=arow, op=Alu.mult)
                nc.vector.tensor_tensor(out=brow, in0=gnb_r, in1=brow, op=Alu.subtract)
                ab_sb = wpool.tile([1, 2 * F], f32, tag="ab_sb", name=f"absb_l{l}")
                nc.vector.tensor_copy(out=ab_sb[:, 0:F], in_=arow)
                nc.vector.tensor_copy(out=ab_sb[:, F : 2 * F], in_=brow)
                nc.sync.dma_start(out=rowab_bounce[par][:], in_=ab_sb[:])
                a_b = wpool.tile([P, F], f32, tag="a_b", name=f"ab_l{l}")
                b_b = wpool.tile([P, F], f32, tag="b_b", name=f"bb_l{l}")
                nc.gpsimd.dma_start(
                    out=a_b[:], in_=rowab_bounce[par][0:1, 0:F].to_broadcast([P, F])
                )
                nc.gpsimd.dma_start(
                    out=b_b[:],
                    in_=rowab_bounce[par][0:1, F : 2 * F].to_broadcast([P, F]),
                )

                xcur = []
                for t in range(nblk):
                    xn_t = wpool.tile([P, F], f32, tag="xn_t", name=f"xnt_l{l}_{t}")
                    nc.vector.tensor_tensor(
                        out=xn_t[:], in0=xnew[t][:], in1=a_b[:], op=Alu.mult
                    )
                    nc.vector.tensor_tensor(
                        out=xn_t[:], in0=xn_t[:], in1=b_b[:], op=Alu.add
                    )
                    relu_t = wpool.tile([P, F], f32, tag="relu_t", name=f"relut_l{l}_{t}")
                    nc.scalar.activation(
                        relu_t[:], xn_t[:], Act.Relu, scale=pr_om[:, 0:1]
                    )
                    xt = xpool.tile([P, F], f32, tag=f"xcur{t}", name=f"xc_l{l}_{t}")
                    nc.vector.scalar_tensor_tensor(
                        out=xt[:],
                        in0=xn_t[:],
                        scalar=pr_col[:, 0:1],
                        in1=relu_t[:],
                        op0=Alu.mult,
                        op1=Alu.add,
                    )
                    xcur.append(xt)
                    if last:
                        nc.sync.dma_start(
                            out=xout_d[t * P : (t + 1) * P, :], in_=xt[:]
                        )

    nc.compile()
    return nc


# --------------------------------------------------------------------------
# Host-side input packing + output assembly.
# --------------------------------------------------------------------------
def pack_inputs(x_full, params, b_chunks, idx_dev, dloc_dev, n_cores=NCORES, d0=D0):
    n_layers = len(params)
    w0 = np.asarray(params[0]["W"], np.float32)
    wr = np.stack(
        [np.asarray(p["W"], np.float32) for p in params[1:]]
    ) if n_layers > 1 else np.zeros((1, F, F), np.float32)
    # a_src/a_dst are linear in xl = x @ W, so fold the attention vectors
    # through W on the host: a = x @ (W @ attflat).
    attm = np.zeros((n_layers, F, 2 * H), np.float32)
    for l, p in enumerate(params):
        a_s = np.asarray(p["att_src"], np.float32)  # [H, C]
        a_d = np.asarray(p["att_dst"], np.float32)
        attflat = np.zeros((F, 2 * H), np.float32)
        for h in range(H):
            attflat[h * C : (h + 1) * C, h] = a_s[h]
            attflat[h * C : (h + 1) * C, H + h] = a_d[h]
        w_l = np.asarray(p["W"], np.float32)
        attm[l, : w_l.shape[0], :] = w_l @ attflat
    bias = np.stack([np.asarray(p["bias"], np.float32) for p in params])
    gnw = np.stack([np.asarray(p["gn_w"], np.float32) for p in params])
    gnb = np.stack([np.asarray(p["gn_b"], np.float32) for p in params])
    gnms = np.stack([np.asarray(p["gn_ms"], np.float32) for p in params])
    prelu = np.array([[np.float32(p["prelu"])] for p in params], np.float32)

    nshard = x_full.shape[0] // n_cores
    in_maps = []
    for k in range(n_cores):
        in_maps.append(
            {
                "x0": np.ascontiguousarray(
                    x_full[k * nshard : (k + 1) * nshard], np.float32
                ),
                "idx": idx_dev[k],
                "dloc": dloc_dev[k],
                "W0": w0,
                "Wr": wr,
                "attm": attm,
                "bias": bias,
                "gnw": gnw,
                "gnb": gnb,
                "gnms": gnms,
                "prelu": prelu,
            }
        )
    return in_maps


LAST_RUN = {}


def kernel(x_spars, edge_index, params, _trace=False):
    from concourse.bass_utils import run_bass_kernel_spmd

    x_spars = np.asarray(x_spars)
    edge_index = np.asarray(edge_index)
    x_full = np.ascontiguousarray(x_spars.reshape(N, D0).astype(np.float32))

    b_chunks, idx_dev, dloc_dev, orig, src_all, dst_all = _plan_edges(edge_index)

    key = ("full", b_chunks)
    if key not in _KERNEL_CACHE:
        _KERNEL_CACHE[key] = build_kernel(NCORES, N, b_chunks)
    nc = _KERNEL_CACHE[key]

    in_maps = pack_inputs(x_full, params, b_chunks, idx_dev, dloc_dev)
    import time as _time

    t0 = _time.time()
    res = run_bass_kernel_spmd(nc, in_maps, list(range(NCORES)), trace=_trace)
    LAST_RUN["wall_s"] = _time.time() - t0
    LAST_RUN["exec_time_ns"] = res.exec_time_ns
    LAST_RUN["res"] = res
    results = res.results

    x_out = np.concatenate([results[k]["xout"] for k in range(NCORES)], axis=0)
    e_tot = src_all.shape[0]
    alpha = np.zeros((e_tot, H), np.float32)
    for k in range(NCORES):
        a_dev = results[k]["aout"]  # [NBLK, P, b_chunks*H]
        a_flat = (
            a_dev.reshape(NBLK, P, b_chunks, H).transpose(0, 2, 1, 3).reshape(-1, H)
        )
        o_flat = orig[k].reshape(-1)
        mask = o_flat >= 0
        alpha[o_flat[mask]] = a_flat[mask]

    edges = np.stack([src_all, dst_all]).astype(edge_index.dtype)
    return x_out, edges, alpha
